# revision 8
# baseline (speedup 1.0000x reference)
"""GCN encoder (nn_Encoder) on 8 TRN2 NeuronCores via Bass/Tile.

Model (PyG GCNConv semantics, eval mode):
    z      = relu(gcn(x, W1, b1))
    mu     = gcn(z, Wmu, bmu)
    logvar = gcn(z, Wlv, blv)
with gcn(x, W, b) = D^-1/2 (A + I) D^-1/2 (x @ W) + b.

Strategy
--------
Because D^-1/2 A D^-1/2 h = D^-1/2 * scatter_add(g[src]) with
g = D^-1/2 * h, pre/post scaling by dinv removes every per-edge
multiply: the edge phase is a pure gather + segment-sum.

Sharding: nodes (padded to 50176 = 8*49*128) are split across 8 cores,
6272 destination rows per core; edges are partitioned by destination
core (hint: "edge partitioning by destination node"). Each core:

  for each window of 128 destination rows: dma_gather rows of the
  scaled feature table T = dinv*x (bf16, batched, 4 SWDGE queues),
  segment-sum via one-hot matmuls (S.T @ G accumulated in PSUM; S is a
  host-built fp8 one-hot of dst-within-window), then apply the weight
  AFTER aggregation (gather commutes with the right-matmul):
  agg_pre -> PE-transpose -> agg_pre.T @ W, and finally
  out = psum*dinv + bias (+relu).  This shards the weight matmul to
  6272 rows/core and avoids materializing x @ W entirely.

mu/logvar layers share the adjacency, so they are fused into one
256-wide layer (Wcat = [Wmu | Wlv]).  The halo exchange of z between
layer 1 and layer 2 happens on host between the two NEFF launches:
NEFF-A outputs dinv*z shards, whose concatenation IS NEFF-B's gather
table.

dma_gather indices are int16, so the 50176-row table is viewed as two
25088-row halves; each core's per-window edge list is grouped by source
half (stream A: src < 25088, stream B: src >= 25088).  SPMD requires
identical per-window tile counts on every core, so counts are padded to
the per-window max over cores with dead edges (one-hot column of
zeros).
"""

import numpy as np
import ml_dtypes

import concourse.bacc as bacc
import concourse.mybir as mybir
import concourse.tile as tile
import concourse.bass_utils as bass_utils

BF16 = ml_dtypes.bfloat16
FP8 = ml_dtypes.float8_e4m3

# ---- problem constants (hardcoded per spec) ----
N = 50000          # nodes
D = 256            # feature width (in = hidden = 2*latent)
C = 8              # cores
WPC = 49           # destination windows (of 128 rows) per core
NPAD = C * WPC * 128   # 50176
SH = WPC * 128         # 6272 rows per core
HALF = NPAD // 2       # 25088 (< int16 max)
GB = 12            # gather batch size, in 128-edge tiles
CH = 1024          # phase-1 node chunk

# test hooks (the grading harness never touches these)
TRACE = False
LAST_EXEC_NS = []
LAST_RESULTS = []


def _enable_trace_shim():
    """Register the NTFF profile hook missing from the trimmed antenv."""
    import sys
    import types

    if "antenv.axon_hooks" in sys.modules:
        return
    mod = types.ModuleType("antenv.axon_hooks")
    mod._hook = None
    mod.set_axon_ntff_profile_hook = lambda h: setattr(mod, "_hook", h)
    mod.get_axon_ntff_profile_hook = lambda: mod._hook
    sys.modules["antenv.axon_hooks"] = mod
    try:
        import antenv

        antenv.axon_hooks = mod
    except ImportError:
        pass
    try:
        from trn_agent_boot.trn_boot import _ntff_profile_via_ctypes

        mod.set_axon_ntff_profile_hook(
            _ntff_profile_via_ctypes("/opt/axon/libaxon_pjrt.so")
        )
    except Exception:
        pass
    bass_utils.upload_artifacts = lambda tmpdir: tmpdir


def _build_layer(TA, TB, relu, out_f32, cfg=None):
    """One GCN layer pass. TA/TB: per-window tile counts for the two
    source halves (len WPC each, same on every core)."""
    from concourse.masks import make_identity

    g = cfg or globals()
    npad, sh, wpc, half, d, gbatch = (
        g["NPAD"], g["SH"], g["WPC"], g["HALF"], g["D"], g["GB"])

    LA = int(np.sum(TA)) * 128
    LB = int(np.sum(TB)) * 128
    f32 = mybir.dt.float32
    bf = mybir.dt.bfloat16
    fp8 = mybir.dt.float8e4

    nc = bacc.Bacc("TRN2", target_bir_lowering=False, num_swdge_queues=4)
    gtab = nc.dram_tensor("gtab", (npad, d), bf, kind="ExternalInput")
    W = nc.dram_tensor("W", (d, d), bf, kind="ExternalInput")
    bt = nc.dram_tensor("bt", (128, d), f32, kind="ExternalInput")
    dw = nc.dram_tensor("dw", (128, wpc), f32, kind="ExternalInput")
    ia = nc.dram_tensor("ia", (128, LA // 16), mybir.dt.int16, kind="ExternalInput")
    ib = nc.dram_tensor("ib", (128, LB // 16), mybir.dt.int16, kind="ExternalInput")
    sa = nc.dram_tensor("sa", (128, LA), fp8, kind="ExternalInput")
    sb = nc.dram_tensor("sb", (128, LB), fp8, kind="ExternalInput")
    selftab = nc.dram_tensor("selftab", (sh, d), bf, kind="ExternalInput")
    out = nc.dram_tensor("out", (sh, d), f32 if out_f32 else bf, kind="ExternalOutput")

    with tile.TileContext(nc) as tc:
        with (
            tc.tile_pool(name="cst", bufs=1) as cst,
            tc.tile_pool(name="gring", bufs=6) as gring,
            tc.tile_pool(name="sring", bufs=6) as sring,
            tc.tile_pool(name="tsb", bufs=4) as tsb,
            tc.tile_pool(name="ep", bufs=4) as ep,
            tc.tile_pool(name="eo", bufs=4) as eo,
            tc.tile_pool(name="ps1", bufs=3, space="PSUM") as ps1p,
            tc.tile_pool(name="pst", bufs=2, space="PSUM") as pstp,
            tc.tile_pool(name="pso", bufs=2, space="PSUM") as psop,
        ):
            w0 = cst.tile([128, d], bf, tag="w0")
            nc.sync.dma_start(out=w0[:], in_=W[0:128, :])
            w1 = cst.tile([128, d], bf, tag="w1")
            nc.sync.dma_start(out=w1[:], in_=W[128:256, :])
            ident = cst.tile([128, 128], bf, tag="ident")
            make_identity(nc, ident[:])
            bt_sb = cst.tile([128, d], f32, tag="bt")
            nc.sync.dma_start(out=bt_sb[:], in_=bt[:])
            dw_sb = cst.tile([128, wpc], f32, tag="dw")
            nc.sync.dma_start(out=dw_sb[:], in_=dw[:])
            ia_sb = cst.tile([128, LA // 16], mybir.dt.int16, tag="ia")
            nc.sync.dma_start(out=ia_sb[:], in_=ia[:])
            ib_sb = cst.tile([128, LB // 16], mybir.dt.int16, tag="ib")
            nc.sync.dma_start(out=ib_sb[:], in_=ib[:])

            qctr = [0]

            class Stream:
                def __init__(self, tl, idx_sb, s_dram, h, tagbase):
                    self.tl = tl
                    self.idx_sb = idx_sb
                    self.s_dram = s_dram
                    self.tab = gtab[h * half:(h + 1) * half, :]
                    self.tag = tagbase
                    self.next_emit = 0
                    self.items = []

                def get(self, pos):
                    while pos >= self.next_emit:
                        t0 = self.next_emit
                        gb_n = min(gbatch, self.tl - t0)
                        gt = gring.tile([128, gbatch, d], bf, tag=self.tag + "g")
                        nc.gpsimd.dma_gather(
                            gt[:, 0:gb_n, :],
                            self.tab,
                            self.idx_sb[:, t0 * 8:(t0 + gb_n) * 8],
                            gb_n * 128,
                            gb_n * 128,
                            d,
                            single_packet=False,
                            queue_num=qctr[0] % 4,
                        )
                        qctr[0] += 1
                        st = sring.tile([128, gbatch * 128], fp8, tag=self.tag + "s")
                        nc.sync.dma_start(
                            out=st[:, 0:gb_n * 128],
                            in_=self.s_dram[:, t0 * 128:(t0 + gb_n) * 128],
                        )
                        for k in range(gb_n):
                            self.items.append((gt, st, k))
                        self.next_emit += gb_n
                    return self.items[pos]

            strA = Stream(LA // 128, ia_sb, sa, 0, "a")
            strB = Stream(LB // 128, ib_sb, sb, 1, "b")

            posA = 0
            posB = 0
            for w in range(wpc):
                ta, tb = int(TA[w]), int(TB[w])
                tot = ta + tb
                ps1 = ps1p.tile([128, d], f32, space="PSUM")
                mmi = 0
                for _ in range(ta):
                    gt, st, slot = strA.get(posA)
                    nc.tensor.matmul(
                        ps1[:], st[:, slot * 128:(slot + 1) * 128], gt[:, slot, :],
                        start=(mmi == 0), stop=(mmi == tot - 1))
                    mmi += 1
                    posA += 1
                for _ in range(tb):
                    gt, st, slot = strB.get(posB)
                    nc.tensor.matmul(
                        ps1[:], st[:, slot * 128:(slot + 1) * 128], gt[:, slot, :],
                        start=(mmi == 0), stop=(mmi == tot - 1))
                    mmi += 1
                    posB += 1

                # agg_pre = psum + own-shard self rows (bf16)
                xw = tsb.tile([128, d], bf, tag="xw")
                nc.sync.dma_start(out=xw[:], in_=selftab[w * 128:(w + 1) * 128, :])
                seg = tsb.tile([128, d], bf, tag="seg")
                if tot > 0:
                    nc.vector.tensor_tensor(
                        out=seg[:], in0=ps1[:], in1=xw[:], op=mybir.AluOpType.add)
                else:
                    nc.vector.tensor_copy(out=seg[:], in_=xw[:])
                pt = pstp.tile([128, d], bf, space="PSUM")
                nc.tensor.transpose(pt[:, 0:128], seg[:, 0:128], ident[:])
                nc.tensor.transpose(pt[:, 128:256], seg[:, 128:256], ident[:])
                tT = tsb.tile([128, d], bf, tag="tT")
                nc.any.tensor_copy(out=tT[:], in_=pt[:])
                po = psop.tile([128, d], f32, space="PSUM")
                nc.tensor.matmul(po[:], tT[:, 0:128], w0[:], start=True, stop=False)
                nc.tensor.matmul(po[:], tT[:, 128:256], w1[:], start=False, stop=True)

                e1 = ep.tile([128, d], f32, tag="e1")
                nc.vector.tensor_scalar(
                    out=e1[:], in0=po[:], scalar1=dw_sb[:, w:w + 1], scalar2=None,
                    op0=mybir.AluOpType.mult)
                if out_f32:
                    o = eo.tile([128, d], f32, tag="o")
                    nc.vector.tensor_tensor(
                        out=o[:], in0=e1[:], in1=bt_sb[:], op=mybir.AluOpType.add)
                else:
                    e2 = ep.tile([128, d], f32, tag="e2")
                    nc.vector.tensor_tensor(
                        out=e2[:], in0=e1[:], in1=bt_sb[:], op=mybir.AluOpType.add)
                    # dinv>0 commutes with relu: dinv*relu(y) == relu(dinv*y)
                    o = eo.tile([128, d], bf, tag="o")
                    nc.scalar.activation(
                        out=o[:], in_=e2[:], func=mybir.ActivationFunctionType.Relu,
                        scale=dw_sb[:, w:w + 1])
                nc.sync.dma_start(out=out[w * 128:(w + 1) * 128, :], in_=o[:])

    nc.compile()
    return nc


def _preprocess(edge_index, cfg=None):
    """Edge partitioning, window->(core,slot) load matching, per-core
    gather-index / one-hot streams."""
    g = cfg or globals()
    n, npad, c_n, wpc, sh, half = g["N"], g["NPAD"], g["C"], g["WPC"], g["SH"], g["HALF"]
    nwin = c_n * wpc

    src = np.asarray(edge_index[0], dtype=np.int64)
    dst = np.asarray(edge_index[1], dtype=np.int64)
    deg = np.bincount(dst, minlength=n).astype(np.float32) + 1.0
    dinv = (1.0 / np.sqrt(deg)).astype(np.float32)
    dinv_pad = np.ones(npad, np.float32)
    dinv_pad[:n] = dinv

    # self-loop contributions are added per-window from the local shard
    # (contiguous rows), not gathered
    h = (src >= half).astype(np.int64)
    gwin = dst >> 7

    # per (global window, half) tile counts
    cnt_gw = np.bincount(gwin * 2 + h, minlength=nwin * 2).reshape(nwin, 2)
    tiles_gw = -(-cnt_gw // 128)

    # assign windows to (core, slot): sort by load, rank-matched groups of
    # C windows share a slot (one per core) -> per-slot max ~ mean
    order_w = np.argsort(-(tiles_gw[:, 0] + tiles_gw[:, 1]), kind="stable")
    win_core = np.empty(nwin, np.int64)
    win_slot = np.empty(nwin, np.int64)
    for s_ in range(wpc):
        grp = order_w[s_ * c_n:(s_ + 1) * c_n]
        win_core[grp] = np.arange(c_n)
        win_slot[grp] = s_
    # slot tile counts = max over the C windows in each slot group
    TA = np.zeros(wpc, np.int64)
    TB = np.zeros(wpc, np.int64)
    for s_ in range(wpc):
        grp = order_w[s_ * c_n:(s_ + 1) * c_n]
        TA[s_] = tiles_gw[grp, 0].max()
        TB[s_] = tiles_gw[grp, 1].max()
    LA = int(TA.sum()) * 128
    LB = int(TB.sum()) * 128
    offA = np.zeros(wpc + 1, np.int64)
    offA[1:] = np.cumsum(TA) * 128
    offB = np.zeros(wpc + 1, np.int64)
    offB[1:] = np.cumsum(TB) * 128

    # sort edges by (core, slot, half)
    key = (win_core[gwin] * wpc + win_slot[gwin]) * 2 + h
    order = np.argsort(key, kind="stable")
    sl = (src - h * half)[order].astype(np.int16)
    drel = (dst & 127)[order].astype(np.float32)
    flat = np.bincount(key, minlength=c_n * wpc * 2)
    gend = np.cumsum(flat)
    gstart = gend - flat

    idxA = np.zeros((c_n, LA), np.int16)
    idxB = np.zeros((c_n, LB), np.int16)
    draA = np.full((c_n, LA), -1.0, np.float32)
    draB = np.full((c_n, LB), -1.0, np.float32)
    for c in range(c_n):
        for wl in range(wpc):
            for hh in (0, 1):
                gi = (c * wpc + wl) * 2 + hh
                g0, g1 = int(gstart[gi]), int(gend[gi])
                nseg = g1 - g0
                off = int((offA if hh == 0 else offB)[wl])
                (idxA if hh == 0 else idxB)[c, off:off + nseg] = sl[g0:g1]
                (draA if hh == 0 else draB)[c, off:off + nseg] = drel[g0:g1]

    m = np.arange(128, dtype=np.float32)

    def dev(idx, dra, L):
        idx_dev = np.tile(np.ascontiguousarray(idx.reshape(-1, 16).T), (8, 1))
        s_dev = np.ascontiguousarray(
            (dra.reshape(-1, 128)[:, :, None] == m).transpose(1, 0, 2)
        ).reshape(128, L).astype(FP8)
        return idx_dev, s_dev

    # slot_to_win[c, s] = global window handled by core c in slot s
    slot_to_win = np.empty((c_n, wpc), np.int64)
    slot_to_win[win_core, win_slot] = np.arange(nwin)

    per_core = []
    for c in range(c_n):
        ia_dev, sa_dev = dev(idxA[c], draA[c], LA)
        ib_dev, sb_dev = dev(idxB[c], draB[c], LB)
        dw_dev = np.ascontiguousarray(dinv_pad[
            (slot_to_win[c][:, None] * 128 + np.arange(128)[None, :]).reshape(-1)
        ].reshape(wpc, 128).T)
        per_core.append(
            {"ia": ia_dev, "ib": ib_dev, "sa": sa_dev, "sb": sb_dev, "dw": dw_dev})
    return dinv_pad, TA, TB, per_core, slot_to_win


_NC_CACHE = {}


def _get_layer_nc(TA, TB, relu, out_f32, cfg=None):
    key = (tuple(int(t) for t in TA), tuple(int(t) for t in TB), relu, out_f32)
    if key not in _NC_CACHE:
        _NC_CACHE[key] = _build_layer(TA, TB, relu, out_f32, cfg=cfg)
    return _NC_CACHE[key]


def _run(nc, in_maps):
    kwargs = {}
    if TRACE:
        _enable_trace_shim()
        kwargs["trace"] = True
    res = bass_utils.run_bass_kernel_spmd(
        nc, in_maps, core_ids=list(range(len(in_maps))), **kwargs)
    if TRACE:
        LAST_EXEC_NS.append(res.exec_time_ns)
        LAST_RESULTS.append(res)
    return res.results


def _kernel_impl(x, edge_index, W1, b1, Wmu, bmu, Wlv, blv, cfg=None):
    g = cfg or globals()
    n, npad, c_n, sh, d = g["N"], g["NPAD"], g["C"], g["SH"], g["D"]

    dinv_pad, TA, TB, per_core, slot_to_win = _preprocess(edge_index, cfg=cfg)

    x = np.asarray(x, dtype=np.float32)
    xs = np.zeros((npad, d), np.float32)
    xs[:n] = x * dinv_pad[:n, None]
    xtab_dev = xs.astype(BF16)

    W1b = np.ascontiguousarray(np.asarray(W1, np.float32)).astype(BF16)
    Wcatb = np.ascontiguousarray(
        np.concatenate([np.asarray(Wmu, np.float32), np.asarray(Wlv, np.float32)],
                       axis=1)).astype(BF16)
    bt1 = np.tile(np.asarray(b1, np.float32)[None, :], (128, 1))
    btc = np.tile(np.concatenate([np.asarray(bmu, np.float32),
                                  np.asarray(blv, np.float32)])[None, :], (128, 1))

    def selftab_for(tab, c):
        rows = (slot_to_win[c][:, None] * 128 + np.arange(128)[None, :]).reshape(-1)
        return np.ascontiguousarray(tab[rows])

    def unpermute(res_list, dtype):
        full = np.empty((npad, d), dtype)
        for c in range(c_n):
            o = np.asarray(res_list[c]["out"])
            rows = (slot_to_win[c][:, None] * 128 + np.arange(128)[None, :]).reshape(-1)
            full[rows] = o
        return full

    ncA = _get_layer_nc(TA, TB, relu=True, out_f32=False, cfg=cfg)
    in_maps_A = [
        {"gtab": xtab_dev, "selftab": selftab_for(xtab_dev, c),
         "W": W1b, "bt": bt1, **per_core[c]} for c in range(c_n)]
    resA = _run(ncA, in_maps_A)
    ztab_dev = unpermute(resA, BF16)

    ncB = _get_layer_nc(TA, TB, relu=False, out_f32=True, cfg=cfg)
    in_maps_B = [
        {"gtab": ztab_dev, "selftab": selftab_for(ztab_dev, c),
         "W": Wcatb, "bt": btc, **per_core[c]} for c in range(c_n)]
    resB = _run(ncB, in_maps_B)
    full = unpermute(resB, np.float32)

    mu = np.ascontiguousarray(full[:n, :d // 2])
    logvar = np.ascontiguousarray(full[:n, d // 2:])
    return mu, logvar


def kernel(x, edge_index, W1, b1, Wmu, bmu, Wlv, blv):
    return _kernel_impl(x, edge_index, W1, b1, Wmu, bmu, Wlv, blv)


# revision 9
# speedup vs baseline: 1.1794x; 1.1794x over previous
"""GCN encoder (nn_Encoder) on 8 TRN2 NeuronCores via Bass/Tile.

Model (PyG GCNConv semantics, eval mode):
    z      = relu(gcn(x, W1, b1))
    mu     = gcn(z, Wmu, bmu)
    logvar = gcn(z, Wlv, blv)
with gcn(x, W, b) = D^-1/2 (A + I) D^-1/2 (x @ W) + b.

Strategy
--------
Because D^-1/2 A D^-1/2 h = D^-1/2 * scatter_add(g[src]) with
g = D^-1/2 * h, pre/post scaling by dinv removes every per-edge
multiply: the edge phase is a pure gather + segment-sum.

Sharding: nodes (padded to 50176 = 8*49*128) are split across 8 cores,
6272 destination rows per core; edges are partitioned by destination
core (hint: "edge partitioning by destination node"). Each core:

  for each window of 128 destination rows: dma_gather rows of the
  scaled feature table T = dinv*x (bf16, batched, 4 SWDGE queues),
  segment-sum via one-hot matmuls (S.T @ G accumulated in PSUM; S is a
  host-built fp8 one-hot of dst-within-window), then apply the weight
  AFTER aggregation (gather commutes with the right-matmul):
  agg_pre -> PE-transpose -> agg_pre.T @ W, and finally
  out = psum*dinv + bias (+relu).  This shards the weight matmul to
  6272 rows/core and avoids materializing x @ W entirely.

mu/logvar layers share the adjacency, so they are fused into one
256-wide layer (Wcat = [Wmu | Wlv]).  The halo exchange of z between
layer 1 and layer 2 happens on host between the two NEFF launches:
NEFF-A outputs dinv*z shards, whose concatenation IS NEFF-B's gather
table.

dma_gather indices are int16, so the 50176-row table is viewed as two
25088-row halves; each core's per-window edge list is grouped by source
half (stream A: src < 25088, stream B: src >= 25088).  SPMD requires
identical per-window tile counts on every core, so counts are padded to
the per-window max over cores with dead edges (one-hot column of
zeros).
"""

import numpy as np
import ml_dtypes

import concourse.bacc as bacc
import concourse.mybir as mybir
import concourse.tile as tile
import concourse.bass_utils as bass_utils

BF16 = ml_dtypes.bfloat16
FP8 = ml_dtypes.float8_e4m3

# ---- problem constants (hardcoded per spec) ----
N = 50000          # nodes
D = 256            # feature width (in = hidden = 2*latent)
C = 8              # cores
WPC = 49           # destination windows (of 128 rows) per core
NPAD = C * WPC * 128   # 50176
SH = WPC * 128         # 6272 rows per core
HALF = NPAD // 2       # 25088 (< int16 max)
GB = 12            # gather batch size, in 128-edge tiles
CH = 1024          # phase-1 node chunk

# test hooks (the grading harness never touches these)
TRACE = False
LAST_EXEC_NS = []
LAST_RESULTS = []


def _enable_trace_shim():
    """Register the NTFF profile hook missing from the trimmed antenv."""
    import sys
    import types

    if "antenv.axon_hooks" in sys.modules:
        return
    mod = types.ModuleType("antenv.axon_hooks")
    mod._hook = None
    mod.set_axon_ntff_profile_hook = lambda h: setattr(mod, "_hook", h)
    mod.get_axon_ntff_profile_hook = lambda: mod._hook
    sys.modules["antenv.axon_hooks"] = mod
    try:
        import antenv

        antenv.axon_hooks = mod
    except ImportError:
        pass
    try:
        from trn_agent_boot.trn_boot import _ntff_profile_via_ctypes

        mod.set_axon_ntff_profile_hook(
            _ntff_profile_via_ctypes("/opt/axon/libaxon_pjrt.so")
        )
    except Exception:
        pass
    bass_utils.upload_artifacts = lambda tmpdir: tmpdir


def _build_layer(TA, TB, relu, out_f32, cfg=None):
    """One GCN layer pass. TA/TB: per-window tile counts for the two
    source halves (len WPC each, same on every core)."""
    from concourse.masks import make_identity

    g = cfg or globals()
    npad, sh, wpc, half, d, gbatch = (
        g["NPAD"], g["SH"], g["WPC"], g["HALF"], g["D"], g["GB"])

    LA = int(np.sum(TA)) * 128
    LB = int(np.sum(TB)) * 128
    f32 = mybir.dt.float32
    bf = mybir.dt.bfloat16

    nc = bacc.Bacc("TRN2", target_bir_lowering=False, num_swdge_queues=4)
    gtab = nc.dram_tensor("gtab", (npad, d), bf, kind="ExternalInput")
    W = nc.dram_tensor("W", (d, d), bf, kind="ExternalInput")
    bt = nc.dram_tensor("bt", (128, d), f32, kind="ExternalInput")
    dw = nc.dram_tensor("dw", (128, wpc), f32, kind="ExternalInput")
    ia = nc.dram_tensor("ia", (128, LA // 16), mybir.dt.int16, kind="ExternalInput")
    ib = nc.dram_tensor("ib", (128, LB // 16), mybir.dt.int16, kind="ExternalInput")
    da = nc.dram_tensor("da", (128, LA // 128), f32, kind="ExternalInput")
    db = nc.dram_tensor("db", (128, LB // 128), f32, kind="ExternalInput")
    io = nc.dram_tensor("io", (128, gbatch * 128), f32, kind="ExternalInput")
    selftab = nc.dram_tensor("selftab", (sh, d), bf, kind="ExternalInput")
    out = nc.dram_tensor("out", (sh, d), f32 if out_f32 else bf, kind="ExternalOutput")

    with tile.TileContext(nc) as tc:
        with (
            tc.tile_pool(name="cst", bufs=1) as cst,
            tc.tile_pool(name="gring", bufs=6) as gring,
            tc.tile_pool(name="sring", bufs=6) as sring,
            tc.tile_pool(name="tsb", bufs=4) as tsb,
            tc.tile_pool(name="ep", bufs=4) as ep,
            tc.tile_pool(name="eo", bufs=4) as eo,
            tc.tile_pool(name="ps1", bufs=3, space="PSUM") as ps1p,
            tc.tile_pool(name="pst", bufs=2, space="PSUM") as pstp,
            tc.tile_pool(name="pso", bufs=2, space="PSUM") as psop,
        ):
            w0 = cst.tile([128, d], bf, tag="w0")
            nc.sync.dma_start(out=w0[:], in_=W[0:128, :])
            w1 = cst.tile([128, d], bf, tag="w1")
            nc.sync.dma_start(out=w1[:], in_=W[128:256, :])
            ident = cst.tile([128, 128], bf, tag="ident")
            make_identity(nc, ident[:])
            bt_sb = cst.tile([128, d], f32, tag="bt")
            nc.sync.dma_start(out=bt_sb[:], in_=bt[:])
            dw_sb = cst.tile([128, wpc], f32, tag="dw")
            nc.sync.dma_start(out=dw_sb[:], in_=dw[:])
            ia_sb = cst.tile([128, LA // 16], mybir.dt.int16, tag="ia")
            nc.sync.dma_start(out=ia_sb[:], in_=ia[:])
            ib_sb = cst.tile([128, LB // 16], mybir.dt.int16, tag="ib")
            nc.sync.dma_start(out=ib_sb[:], in_=ib[:])
            da_sb = cst.tile([128, LA // 128], f32, tag="da")
            nc.sync.dma_start(out=da_sb[:], in_=da[:])
            db_sb = cst.tile([128, LB // 128], f32, tag="db")
            nc.sync.dma_start(out=db_sb[:], in_=db[:])
            io_sb = cst.tile([128, gbatch * 128], f32, tag="io")
            nc.sync.dma_start(out=io_sb[:], in_=io[:])

            qctr = [0]

            class Stream:
                def __init__(self, tl, idx_sb, d_sb, h, tagbase):
                    self.tl = tl
                    self.idx_sb = idx_sb
                    self.d_sb = d_sb
                    self.tab = gtab[h * half:(h + 1) * half, :]
                    self.tag = tagbase
                    self.next_emit = 0
                    self.items = []

                def get(self, pos):
                    while pos >= self.next_emit:
                        t0 = self.next_emit
                        gb_n = min(gbatch, self.tl - t0)
                        gt = gring.tile([128, gbatch, d], bf, tag=self.tag + "g")
                        nc.gpsimd.dma_gather(
                            gt[:, 0:gb_n, :],
                            self.tab,
                            self.idx_sb[:, t0 * 8:(t0 + gb_n) * 8],
                            gb_n * 128,
                            gb_n * 128,
                            d,
                            single_packet=False,
                            queue_num=qctr[0] % 4,
                        )
                        qctr[0] += 1
                        st = sring.tile([128, gbatch * 128], bf, tag=self.tag + "s")
                        nc.vector.tensor_tensor(
                            out=st[:, 0:gb_n * 128].rearrange(
                                "p (a b) -> p a b", b=128),
                            in0=io_sb[:, 0:gb_n * 128].rearrange(
                                "p (a b) -> p a b", b=128),
                            in1=self.d_sb[:, t0:t0 + gb_n].to_broadcast(
                                [128, gb_n, 128]),
                            op=mybir.AluOpType.is_equal,
                        )
                        for k in range(gb_n):
                            self.items.append((gt, st, k))
                        self.next_emit += gb_n
                    return self.items[pos]

            strA = Stream(LA // 128, ia_sb, da_sb, 0, "a")
            strB = Stream(LB // 128, ib_sb, db_sb, 1, "b")

            posA = 0
            posB = 0
            for w in range(wpc):
                ta, tb = int(TA[w]), int(TB[w])
                tot = ta + tb
                ps1 = ps1p.tile([128, d], f32, space="PSUM")
                mmi = 0
                for _ in range(ta):
                    gt, st, slot = strA.get(posA)
                    nc.tensor.matmul(
                        ps1[:], st[:, slot * 128:(slot + 1) * 128], gt[:, slot, :],
                        start=(mmi == 0), stop=(mmi == tot - 1))
                    mmi += 1
                    posA += 1
                for _ in range(tb):
                    gt, st, slot = strB.get(posB)
                    nc.tensor.matmul(
                        ps1[:], st[:, slot * 128:(slot + 1) * 128], gt[:, slot, :],
                        start=(mmi == 0), stop=(mmi == tot - 1))
                    mmi += 1
                    posB += 1

                # agg_pre = psum + own-shard self rows (bf16)
                xw = tsb.tile([128, d], bf, tag="xw")
                nc.sync.dma_start(out=xw[:], in_=selftab[w * 128:(w + 1) * 128, :])
                seg = tsb.tile([128, d], bf, tag="seg")
                if tot > 0:
                    nc.vector.tensor_tensor(
                        out=seg[:], in0=ps1[:], in1=xw[:], op=mybir.AluOpType.add)
                else:
                    nc.vector.tensor_copy(out=seg[:], in_=xw[:])
                pt = pstp.tile([128, d], bf, space="PSUM")
                nc.tensor.transpose(pt[:, 0:128], seg[:, 0:128], ident[:])
                nc.tensor.transpose(pt[:, 128:256], seg[:, 128:256], ident[:])
                tT = tsb.tile([128, d], bf, tag="tT")
                nc.any.tensor_copy(out=tT[:], in_=pt[:])
                po = psop.tile([128, d], f32, space="PSUM")
                nc.tensor.matmul(po[:], tT[:, 0:128], w0[:], start=True, stop=False)
                nc.tensor.matmul(po[:], tT[:, 128:256], w1[:], start=False, stop=True)

                e1 = ep.tile([128, d], f32, tag="e1")
                nc.vector.tensor_scalar(
                    out=e1[:], in0=po[:], scalar1=dw_sb[:, w:w + 1], scalar2=None,
                    op0=mybir.AluOpType.mult)
                if out_f32:
                    o = eo.tile([128, d], f32, tag="o")
                    nc.vector.tensor_tensor(
                        out=o[:], in0=e1[:], in1=bt_sb[:], op=mybir.AluOpType.add)
                else:
                    e2 = ep.tile([128, d], f32, tag="e2")
                    nc.vector.tensor_tensor(
                        out=e2[:], in0=e1[:], in1=bt_sb[:], op=mybir.AluOpType.add)
                    # dinv>0 commutes with relu: dinv*relu(y) == relu(dinv*y)
                    o = eo.tile([128, d], bf, tag="o")
                    nc.scalar.activation(
                        out=o[:], in_=e2[:], func=mybir.ActivationFunctionType.Relu,
                        scale=dw_sb[:, w:w + 1])
                nc.sync.dma_start(out=out[w * 128:(w + 1) * 128, :], in_=o[:])

    nc.compile()
    return nc


def _preprocess(edge_index, cfg=None):
    """Edge partitioning, window->(core,slot) load matching, per-core
    gather-index / one-hot streams."""
    g = cfg or globals()
    n, npad, c_n, wpc, sh, half = g["N"], g["NPAD"], g["C"], g["WPC"], g["SH"], g["HALF"]
    nwin = c_n * wpc

    src = np.asarray(edge_index[0], dtype=np.int64)
    dst = np.asarray(edge_index[1], dtype=np.int64)
    deg = np.bincount(dst, minlength=n).astype(np.float32) + 1.0
    dinv = (1.0 / np.sqrt(deg)).astype(np.float32)
    dinv_pad = np.ones(npad, np.float32)
    dinv_pad[:n] = dinv

    # self-loop contributions are added per-window from the local shard
    # (contiguous rows), not gathered
    h = (src >= half).astype(np.int64)
    gwin = dst >> 7

    # per (global window, half) tile counts
    cnt_gw = np.bincount(gwin * 2 + h, minlength=nwin * 2).reshape(nwin, 2)
    tiles_gw = -(-cnt_gw // 128)

    # assign windows to (core, slot): sort by load, rank-matched groups of
    # C windows share a slot (one per core) -> per-slot max ~ mean
    order_w = np.argsort(-(tiles_gw[:, 0] + tiles_gw[:, 1]), kind="stable")
    win_core = np.empty(nwin, np.int64)
    win_slot = np.empty(nwin, np.int64)
    for s_ in range(wpc):
        grp = order_w[s_ * c_n:(s_ + 1) * c_n]
        win_core[grp] = np.arange(c_n)
        win_slot[grp] = s_
    # slot tile counts = max over the C windows in each slot group
    TA = np.zeros(wpc, np.int64)
    TB = np.zeros(wpc, np.int64)
    for s_ in range(wpc):
        grp = order_w[s_ * c_n:(s_ + 1) * c_n]
        TA[s_] = tiles_gw[grp, 0].max()
        TB[s_] = tiles_gw[grp, 1].max()
    LA = int(TA.sum()) * 128
    LB = int(TB.sum()) * 128
    offA = np.zeros(wpc + 1, np.int64)
    offA[1:] = np.cumsum(TA) * 128
    offB = np.zeros(wpc + 1, np.int64)
    offB[1:] = np.cumsum(TB) * 128

    # sort edges by (core, slot, half)
    key = (win_core[gwin] * wpc + win_slot[gwin]) * 2 + h
    order = np.argsort(key, kind="stable")
    sl = (src - h * half)[order].astype(np.int16)
    drel = (dst & 127)[order].astype(np.float32)
    flat = np.bincount(key, minlength=c_n * wpc * 2)
    gend = np.cumsum(flat)
    gstart = gend - flat

    idxA = np.zeros((c_n, LA), np.int16)
    idxB = np.zeros((c_n, LB), np.int16)
    draA = np.full((c_n, LA), -1.0, np.float32)
    draB = np.full((c_n, LB), -1.0, np.float32)
    for c in range(c_n):
        for wl in range(wpc):
            for hh in (0, 1):
                gi = (c * wpc + wl) * 2 + hh
                g0, g1 = int(gstart[gi]), int(gend[gi])
                nseg = g1 - g0
                off = int((offA if hh == 0 else offB)[wl])
                (idxA if hh == 0 else idxB)[c, off:off + nseg] = sl[g0:g1]
                (draA if hh == 0 else draB)[c, off:off + nseg] = drel[g0:g1]

    def dev(idx, dra, L):
        idx_dev = np.tile(np.ascontiguousarray(idx.reshape(-1, 16).T), (8, 1))
        d_dev = np.ascontiguousarray(dra.reshape(-1, 128).T)
        return idx_dev, d_dev

    # slot_to_win[c, s] = global window handled by core c in slot s
    slot_to_win = np.empty((c_n, wpc), np.int64)
    slot_to_win[win_core, win_slot] = np.arange(nwin)

    io_dev = np.tile(np.arange(128, dtype=np.float32), (128, g["GB"]))
    per_core = []
    for c in range(c_n):
        ia_dev, da_dev = dev(idxA[c], draA[c], LA)
        ib_dev, db_dev = dev(idxB[c], draB[c], LB)
        dw_dev = np.ascontiguousarray(dinv_pad[
            (slot_to_win[c][:, None] * 128 + np.arange(128)[None, :]).reshape(-1)
        ].reshape(wpc, 128).T)
        per_core.append(
            {"ia": ia_dev, "ib": ib_dev, "da": da_dev, "db": db_dev,
             "io": io_dev, "dw": dw_dev})
    return dinv_pad, TA, TB, per_core, slot_to_win


_NC_CACHE = {}


def _get_layer_nc(TA, TB, relu, out_f32, cfg=None):
    key = (tuple(int(t) for t in TA), tuple(int(t) for t in TB), relu, out_f32)
    if key not in _NC_CACHE:
        _NC_CACHE[key] = _build_layer(TA, TB, relu, out_f32, cfg=cfg)
    return _NC_CACHE[key]


def _run(nc, in_maps):
    kwargs = {}
    if TRACE:
        _enable_trace_shim()
        kwargs["trace"] = True
    res = bass_utils.run_bass_kernel_spmd(
        nc, in_maps, core_ids=list(range(len(in_maps))), **kwargs)
    if TRACE:
        LAST_EXEC_NS.append(res.exec_time_ns)
        LAST_RESULTS.append(res)
    return res.results


def _kernel_impl(x, edge_index, W1, b1, Wmu, bmu, Wlv, blv, cfg=None):
    g = cfg or globals()
    n, npad, c_n, sh, d = g["N"], g["NPAD"], g["C"], g["SH"], g["D"]

    dinv_pad, TA, TB, per_core, slot_to_win = _preprocess(edge_index, cfg=cfg)

    x = np.asarray(x, dtype=np.float32)
    xs = np.zeros((npad, d), np.float32)
    xs[:n] = x * dinv_pad[:n, None]
    xtab_dev = xs.astype(BF16)

    W1b = np.ascontiguousarray(np.asarray(W1, np.float32)).astype(BF16)
    Wcatb = np.ascontiguousarray(
        np.concatenate([np.asarray(Wmu, np.float32), np.asarray(Wlv, np.float32)],
                       axis=1)).astype(BF16)
    bt1 = np.tile(np.asarray(b1, np.float32)[None, :], (128, 1))
    btc = np.tile(np.concatenate([np.asarray(bmu, np.float32),
                                  np.asarray(blv, np.float32)])[None, :], (128, 1))

    def selftab_for(tab, c):
        rows = (slot_to_win[c][:, None] * 128 + np.arange(128)[None, :]).reshape(-1)
        return np.ascontiguousarray(tab[rows])

    def unpermute(res_list, dtype):
        full = np.empty((npad, d), dtype)
        for c in range(c_n):
            o = np.asarray(res_list[c]["out"])
            rows = (slot_to_win[c][:, None] * 128 + np.arange(128)[None, :]).reshape(-1)
            full[rows] = o
        return full

    ncA = _get_layer_nc(TA, TB, relu=True, out_f32=False, cfg=cfg)
    in_maps_A = [
        {"gtab": xtab_dev, "selftab": selftab_for(xtab_dev, c),
         "W": W1b, "bt": bt1, **per_core[c]} for c in range(c_n)]
    resA = _run(ncA, in_maps_A)
    ztab_dev = unpermute(resA, BF16)

    ncB = _get_layer_nc(TA, TB, relu=False, out_f32=True, cfg=cfg)
    in_maps_B = [
        {"gtab": ztab_dev, "selftab": selftab_for(ztab_dev, c),
         "W": Wcatb, "bt": btc, **per_core[c]} for c in range(c_n)]
    resB = _run(ncB, in_maps_B)
    full = unpermute(resB, np.float32)

    mu = np.ascontiguousarray(full[:n, :d // 2])
    logvar = np.ascontiguousarray(full[:n, d // 2:])
    return mu, logvar


def kernel(x, edge_index, W1, b1, Wmu, bmu, Wlv, blv):
    return _kernel_impl(x, edge_index, W1, b1, Wmu, bmu, Wlv, blv)


# revision 10
# speedup vs baseline: 1.2092x; 1.0252x over previous
"""GCN encoder (nn_Encoder) on 8 TRN2 NeuronCores via Bass/Tile.

Model (PyG GCNConv semantics, eval mode):
    z      = relu(gcn(x, W1, b1))
    mu     = gcn(z, Wmu, bmu)
    logvar = gcn(z, Wlv, blv)
with gcn(x, W, b) = D^-1/2 (A + I) D^-1/2 (x @ W) + b.

Strategy
--------
Because D^-1/2 A D^-1/2 h = D^-1/2 * scatter_add(g[src]) with
g = D^-1/2 * h, pre/post scaling by dinv removes every per-edge
multiply: the edge phase is a pure gather + segment-sum.

Sharding: nodes (padded to 50176 = 8*49*128) are split across 8 cores,
6272 destination rows per core; edges are partitioned by destination
core (hint: "edge partitioning by destination node"). Each core:

  for each window of 128 destination rows: dma_gather rows of the
  scaled feature table T = dinv*x (bf16, batched, 4 SWDGE queues),
  segment-sum via one-hot matmuls (S.T @ G accumulated in PSUM; S is a
  host-built fp8 one-hot of dst-within-window), then apply the weight
  AFTER aggregation (gather commutes with the right-matmul):
  agg_pre -> PE-transpose -> agg_pre.T @ W, and finally
  out = psum*dinv + bias (+relu).  This shards the weight matmul to
  6272 rows/core and avoids materializing x @ W entirely.

mu/logvar layers share the adjacency, so they are fused into one
256-wide layer (Wcat = [Wmu | Wlv]).  The halo exchange of z between
layer 1 and layer 2 happens on host between the two NEFF launches:
NEFF-A outputs dinv*z shards, whose concatenation IS NEFF-B's gather
table.

dma_gather indices are int16, so the 50176-row table is viewed as two
25088-row halves; each core's per-window edge list is grouped by source
half (stream A: src < 25088, stream B: src >= 25088).  SPMD requires
identical per-window tile counts on every core, so counts are padded to
the per-window max over cores with dead edges (one-hot column of
zeros).
"""

import numpy as np
import ml_dtypes

import concourse.bacc as bacc
import concourse.mybir as mybir
import concourse.tile as tile
import concourse.bass_utils as bass_utils

BF16 = ml_dtypes.bfloat16
FP8 = ml_dtypes.float8_e4m3

# ---- problem constants (hardcoded per spec) ----
N = 50000          # nodes
D = 256            # feature width (in = hidden = 2*latent)
C = 8              # cores
WPC = 49           # destination windows (of 128 rows) per core
NPAD = C * WPC * 128   # 50176
SH = WPC * 128         # 6272 rows per core
HALF = NPAD // 2       # 25088 (< int16 max)
GB = 16            # gather batch size, in 128-edge tiles
CH = 1024          # phase-1 node chunk

# test hooks (the grading harness never touches these)
TRACE = False
LAST_EXEC_NS = []
LAST_RESULTS = []


def _enable_trace_shim():
    """Register the NTFF profile hook missing from the trimmed antenv."""
    import sys
    import types

    if "antenv.axon_hooks" in sys.modules:
        return
    mod = types.ModuleType("antenv.axon_hooks")
    mod._hook = None
    mod.set_axon_ntff_profile_hook = lambda h: setattr(mod, "_hook", h)
    mod.get_axon_ntff_profile_hook = lambda: mod._hook
    sys.modules["antenv.axon_hooks"] = mod
    try:
        import antenv

        antenv.axon_hooks = mod
    except ImportError:
        pass
    try:
        from trn_agent_boot.trn_boot import _ntff_profile_via_ctypes

        mod.set_axon_ntff_profile_hook(
            _ntff_profile_via_ctypes("/opt/axon/libaxon_pjrt.so")
        )
    except Exception:
        pass
    bass_utils.upload_artifacts = lambda tmpdir: tmpdir


def _build_layer(TA, TB, relu, out_f32, cfg=None):
    """One GCN layer pass. TA/TB: per-window tile counts for the two
    source halves (len WPC each, same on every core)."""
    from concourse.masks import make_identity

    g = cfg or globals()
    npad, sh, wpc, half, d, gbatch = (
        g["NPAD"], g["SH"], g["WPC"], g["HALF"], g["D"], g["GB"])

    LA = int(np.sum(TA)) * 128
    LB = int(np.sum(TB)) * 128
    f32 = mybir.dt.float32
    bf = mybir.dt.bfloat16

    nc = bacc.Bacc("TRN2", target_bir_lowering=False, num_swdge_queues=4)
    gtab = nc.dram_tensor("gtab", (npad, d), bf, kind="ExternalInput")
    W = nc.dram_tensor("W", (d, d), bf, kind="ExternalInput")
    bt = nc.dram_tensor("bt", (128, d), f32, kind="ExternalInput")
    dw = nc.dram_tensor("dw", (128, wpc), f32, kind="ExternalInput")
    ia = nc.dram_tensor("ia", (128, LA // 16), mybir.dt.int16, kind="ExternalInput")
    ib = nc.dram_tensor("ib", (128, LB // 16), mybir.dt.int16, kind="ExternalInput")
    da = nc.dram_tensor("da", (128, LA // 128), f32, kind="ExternalInput")
    db = nc.dram_tensor("db", (128, LB // 128), f32, kind="ExternalInput")
    io = nc.dram_tensor("io", (128, gbatch * 128), f32, kind="ExternalInput")
    selftab = nc.dram_tensor("selftab", (sh, d), bf, kind="ExternalInput")
    out = nc.dram_tensor("out", (sh, d), f32 if out_f32 else bf, kind="ExternalOutput")

    with tile.TileContext(nc) as tc:
        with (
            tc.tile_pool(name="cst", bufs=1) as cst,
            tc.tile_pool(name="gring", bufs=6) as gring,
            tc.tile_pool(name="sring", bufs=6) as sring,
            tc.tile_pool(name="tsb", bufs=4) as tsb,
            tc.tile_pool(name="ep", bufs=4) as ep,
            tc.tile_pool(name="eo", bufs=4) as eo,
            tc.tile_pool(name="ps1", bufs=3, space="PSUM") as ps1p,
            tc.tile_pool(name="pst", bufs=2, space="PSUM") as pstp,
            tc.tile_pool(name="pso", bufs=2, space="PSUM") as psop,
        ):
            w0 = cst.tile([128, d], bf, tag="w0")
            nc.sync.dma_start(out=w0[:], in_=W[0:128, :])
            w1 = cst.tile([128, d], bf, tag="w1")
            nc.sync.dma_start(out=w1[:], in_=W[128:256, :])
            ident = cst.tile([128, 128], bf, tag="ident")
            make_identity(nc, ident[:])
            bt_sb = cst.tile([128, d], f32, tag="bt")
            nc.sync.dma_start(out=bt_sb[:], in_=bt[:])
            dw_sb = cst.tile([128, wpc], f32, tag="dw")
            nc.sync.dma_start(out=dw_sb[:], in_=dw[:])
            ia_sb = cst.tile([128, LA // 16], mybir.dt.int16, tag="ia")
            nc.sync.dma_start(out=ia_sb[:], in_=ia[:])
            ib_sb = cst.tile([128, LB // 16], mybir.dt.int16, tag="ib")
            nc.sync.dma_start(out=ib_sb[:], in_=ib[:])
            da_sb = cst.tile([128, LA // 128], f32, tag="da")
            nc.sync.dma_start(out=da_sb[:], in_=da[:])
            db_sb = cst.tile([128, LB // 128], f32, tag="db")
            nc.sync.dma_start(out=db_sb[:], in_=db[:])
            io_sb = cst.tile([128, gbatch * 128], f32, tag="io")
            nc.sync.dma_start(out=io_sb[:], in_=io[:])

            qctr = [0]

            class Stream:
                def __init__(self, tl, idx_sb, d_sb, h, tagbase):
                    self.tl = tl
                    self.idx_sb = idx_sb
                    self.d_sb = d_sb
                    self.tab = gtab[h * half:(h + 1) * half, :]
                    self.tag = tagbase
                    self.next_emit = 0
                    self.items = []

                def get(self, pos):
                    while pos >= self.next_emit:
                        t0 = self.next_emit
                        gb_n = min(gbatch, self.tl - t0)
                        gt = gring.tile([128, gbatch, d], bf, tag=self.tag + "g")
                        nc.gpsimd.dma_gather(
                            gt[:, 0:gb_n, :],
                            self.tab,
                            self.idx_sb[:, t0 * 8:(t0 + gb_n) * 8],
                            gb_n * 128,
                            gb_n * 128,
                            d,
                            single_packet=False,
                            queue_num=qctr[0] % 4,
                        )
                        qctr[0] += 1
                        st = sring.tile([128, gbatch * 128], bf, tag=self.tag + "s")
                        nc.vector.tensor_tensor(
                            out=st[:, 0:gb_n * 128].rearrange(
                                "p (a b) -> p a b", b=128),
                            in0=io_sb[:, 0:gb_n * 128].rearrange(
                                "p (a b) -> p a b", b=128),
                            in1=self.d_sb[:, t0:t0 + gb_n].to_broadcast(
                                [128, gb_n, 128]),
                            op=mybir.AluOpType.is_equal,
                        )
                        for k in range(gb_n):
                            self.items.append((gt, st, k))
                        self.next_emit += gb_n
                    return self.items[pos]

            strA = Stream(LA // 128, ia_sb, da_sb, 0, "a")
            strB = Stream(LB // 128, ib_sb, db_sb, 1, "b")

            posA = 0
            posB = 0
            for w in range(wpc):
                ta, tb = int(TA[w]), int(TB[w])
                tot = ta + tb
                ps1 = ps1p.tile([128, d], f32, space="PSUM")
                mmi = 0
                for _ in range(ta):
                    gt, st, slot = strA.get(posA)
                    nc.tensor.matmul(
                        ps1[:], st[:, slot * 128:(slot + 1) * 128], gt[:, slot, :],
                        start=(mmi == 0), stop=(mmi == tot - 1))
                    mmi += 1
                    posA += 1
                for _ in range(tb):
                    gt, st, slot = strB.get(posB)
                    nc.tensor.matmul(
                        ps1[:], st[:, slot * 128:(slot + 1) * 128], gt[:, slot, :],
                        start=(mmi == 0), stop=(mmi == tot - 1))
                    mmi += 1
                    posB += 1

                # agg_pre = psum + own-shard self rows (bf16)
                xw = tsb.tile([128, d], bf, tag="xw")
                nc.sync.dma_start(out=xw[:], in_=selftab[w * 128:(w + 1) * 128, :])
                seg = tsb.tile([128, d], bf, tag="seg")
                if tot > 0:
                    nc.vector.tensor_tensor(
                        out=seg[:], in0=ps1[:], in1=xw[:], op=mybir.AluOpType.add)
                else:
                    nc.vector.tensor_copy(out=seg[:], in_=xw[:])
                pt = pstp.tile([128, d], bf, space="PSUM")
                nc.tensor.transpose(pt[:, 0:128], seg[:, 0:128], ident[:])
                nc.tensor.transpose(pt[:, 128:256], seg[:, 128:256], ident[:])
                tT = tsb.tile([128, d], bf, tag="tT")
                nc.any.tensor_copy(out=tT[:], in_=pt[:])
                po = psop.tile([128, d], f32, space="PSUM")
                nc.tensor.matmul(po[:], tT[:, 0:128], w0[:], start=True, stop=False)
                nc.tensor.matmul(po[:], tT[:, 128:256], w1[:], start=False, stop=True)

                e1 = ep.tile([128, d], f32, tag="e1")
                nc.vector.tensor_scalar(
                    out=e1[:], in0=po[:], scalar1=dw_sb[:, w:w + 1], scalar2=None,
                    op0=mybir.AluOpType.mult)
                if out_f32:
                    o = eo.tile([128, d], f32, tag="o")
                    nc.vector.tensor_tensor(
                        out=o[:], in0=e1[:], in1=bt_sb[:], op=mybir.AluOpType.add)
                else:
                    e2 = ep.tile([128, d], f32, tag="e2")
                    nc.vector.tensor_tensor(
                        out=e2[:], in0=e1[:], in1=bt_sb[:], op=mybir.AluOpType.add)
                    # dinv>0 commutes with relu: dinv*relu(y) == relu(dinv*y)
                    o = eo.tile([128, d], bf, tag="o")
                    nc.scalar.activation(
                        out=o[:], in_=e2[:], func=mybir.ActivationFunctionType.Relu,
                        scale=dw_sb[:, w:w + 1])
                nc.sync.dma_start(out=out[w * 128:(w + 1) * 128, :], in_=o[:])

    nc.compile()
    return nc


def _preprocess(edge_index, cfg=None):
    """Edge partitioning, window->(core,slot) load matching, per-core
    gather-index / one-hot streams."""
    g = cfg or globals()
    n, npad, c_n, wpc, sh, half = g["N"], g["NPAD"], g["C"], g["WPC"], g["SH"], g["HALF"]
    nwin = c_n * wpc

    src = np.asarray(edge_index[0], dtype=np.int64)
    dst = np.asarray(edge_index[1], dtype=np.int64)
    deg = np.bincount(dst, minlength=n).astype(np.float32) + 1.0
    dinv = (1.0 / np.sqrt(deg)).astype(np.float32)
    dinv_pad = np.ones(npad, np.float32)
    dinv_pad[:n] = dinv

    # self-loop contributions are added per-window from the local shard
    # (contiguous rows), not gathered
    h = (src >= half).astype(np.int64)
    gwin = dst >> 7

    # per (global window, half) tile counts
    cnt_gw = np.bincount(gwin * 2 + h, minlength=nwin * 2).reshape(nwin, 2)
    tiles_gw = -(-cnt_gw // 128)

    # assign windows to (core, slot): sort by load, rank-matched groups of
    # C windows share a slot (one per core) -> per-slot max ~ mean
    order_w = np.argsort(-(tiles_gw[:, 0] + tiles_gw[:, 1]), kind="stable")
    win_core = np.empty(nwin, np.int64)
    win_slot = np.empty(nwin, np.int64)
    for s_ in range(wpc):
        grp = order_w[s_ * c_n:(s_ + 1) * c_n]
        win_core[grp] = np.arange(c_n)
        win_slot[grp] = s_
    # slot tile counts = max over the C windows in each slot group
    TA = np.zeros(wpc, np.int64)
    TB = np.zeros(wpc, np.int64)
    for s_ in range(wpc):
        grp = order_w[s_ * c_n:(s_ + 1) * c_n]
        TA[s_] = tiles_gw[grp, 0].max()
        TB[s_] = tiles_gw[grp, 1].max()
    LA = int(TA.sum()) * 128
    LB = int(TB.sum()) * 128
    offA = np.zeros(wpc + 1, np.int64)
    offA[1:] = np.cumsum(TA) * 128
    offB = np.zeros(wpc + 1, np.int64)
    offB[1:] = np.cumsum(TB) * 128

    # sort edges by (core, slot, half)
    key = (win_core[gwin] * wpc + win_slot[gwin]) * 2 + h
    order = np.argsort(key, kind="stable")
    sl = (src - h * half)[order].astype(np.int16)
    drel = (dst & 127)[order].astype(np.float32)
    flat = np.bincount(key, minlength=c_n * wpc * 2)
    gend = np.cumsum(flat)
    gstart = gend - flat

    idxA = np.zeros((c_n, LA), np.int16)
    idxB = np.zeros((c_n, LB), np.int16)
    draA = np.full((c_n, LA), -1.0, np.float32)
    draB = np.full((c_n, LB), -1.0, np.float32)
    for c in range(c_n):
        for wl in range(wpc):
            for hh in (0, 1):
                gi = (c * wpc + wl) * 2 + hh
                g0, g1 = int(gstart[gi]), int(gend[gi])
                nseg = g1 - g0
                off = int((offA if hh == 0 else offB)[wl])
                (idxA if hh == 0 else idxB)[c, off:off + nseg] = sl[g0:g1]
                (draA if hh == 0 else draB)[c, off:off + nseg] = drel[g0:g1]

    def dev(idx, dra, L):
        idx_dev = np.tile(np.ascontiguousarray(idx.reshape(-1, 16).T), (8, 1))
        d_dev = np.ascontiguousarray(dra.reshape(-1, 128).T)
        return idx_dev, d_dev

    # slot_to_win[c, s] = global window handled by core c in slot s
    slot_to_win = np.empty((c_n, wpc), np.int64)
    slot_to_win[win_core, win_slot] = np.arange(nwin)

    io_dev = np.tile(np.arange(128, dtype=np.float32), (128, g["GB"]))
    per_core = []
    for c in range(c_n):
        ia_dev, da_dev = dev(idxA[c], draA[c], LA)
        ib_dev, db_dev = dev(idxB[c], draB[c], LB)
        dw_dev = np.ascontiguousarray(dinv_pad[
            (slot_to_win[c][:, None] * 128 + np.arange(128)[None, :]).reshape(-1)
        ].reshape(wpc, 128).T)
        per_core.append(
            {"ia": ia_dev, "ib": ib_dev, "da": da_dev, "db": db_dev,
             "io": io_dev, "dw": dw_dev})
    return dinv_pad, TA, TB, per_core, slot_to_win


_NC_CACHE = {}


def _get_layer_nc(TA, TB, relu, out_f32, cfg=None):
    key = (tuple(int(t) for t in TA), tuple(int(t) for t in TB), relu, out_f32)
    if key not in _NC_CACHE:
        _NC_CACHE[key] = _build_layer(TA, TB, relu, out_f32, cfg=cfg)
    return _NC_CACHE[key]


def _run(nc, in_maps):
    kwargs = {}
    if TRACE:
        _enable_trace_shim()
        kwargs["trace"] = True
    res = bass_utils.run_bass_kernel_spmd(
        nc, in_maps, core_ids=list(range(len(in_maps))), **kwargs)
    if TRACE:
        LAST_EXEC_NS.append(res.exec_time_ns)
        LAST_RESULTS.append(res)
    return res.results


def _kernel_impl(x, edge_index, W1, b1, Wmu, bmu, Wlv, blv, cfg=None):
    g = cfg or globals()
    n, npad, c_n, sh, d = g["N"], g["NPAD"], g["C"], g["SH"], g["D"]

    dinv_pad, TA, TB, per_core, slot_to_win = _preprocess(edge_index, cfg=cfg)

    x = np.asarray(x, dtype=np.float32)
    xs = np.zeros((npad, d), np.float32)
    xs[:n] = x * dinv_pad[:n, None]
    xtab_dev = xs.astype(BF16)

    W1b = np.ascontiguousarray(np.asarray(W1, np.float32)).astype(BF16)
    Wcatb = np.ascontiguousarray(
        np.concatenate([np.asarray(Wmu, np.float32), np.asarray(Wlv, np.float32)],
                       axis=1)).astype(BF16)
    bt1 = np.tile(np.asarray(b1, np.float32)[None, :], (128, 1))
    btc = np.tile(np.concatenate([np.asarray(bmu, np.float32),
                                  np.asarray(blv, np.float32)])[None, :], (128, 1))

    def selftab_for(tab, c):
        rows = (slot_to_win[c][:, None] * 128 + np.arange(128)[None, :]).reshape(-1)
        return np.ascontiguousarray(tab[rows])

    def unpermute(res_list, dtype):
        full = np.empty((npad, d), dtype)
        for c in range(c_n):
            o = np.asarray(res_list[c]["out"])
            rows = (slot_to_win[c][:, None] * 128 + np.arange(128)[None, :]).reshape(-1)
            full[rows] = o
        return full

    ncA = _get_layer_nc(TA, TB, relu=True, out_f32=False, cfg=cfg)
    in_maps_A = [
        {"gtab": xtab_dev, "selftab": selftab_for(xtab_dev, c),
         "W": W1b, "bt": bt1, **per_core[c]} for c in range(c_n)]
    resA = _run(ncA, in_maps_A)
    ztab_dev = unpermute(resA, BF16)

    ncB = _get_layer_nc(TA, TB, relu=False, out_f32=True, cfg=cfg)
    in_maps_B = [
        {"gtab": ztab_dev, "selftab": selftab_for(ztab_dev, c),
         "W": Wcatb, "bt": btc, **per_core[c]} for c in range(c_n)]
    resB = _run(ncB, in_maps_B)
    full = unpermute(resB, np.float32)

    mu = np.ascontiguousarray(full[:n, :d // 2])
    logvar = np.ascontiguousarray(full[:n, d // 2:])
    return mu, logvar


def kernel(x, edge_index, W1, b1, Wmu, bmu, Wlv, blv):
    return _kernel_impl(x, edge_index, W1, b1, Wmu, bmu, Wlv, blv)
